# revision 1
# baseline (speedup 1.0000x reference)
"""Data-parallel EntityAttentionRNNAgent on 8 NeuronCores.

Shards the batch dim (bs=32 -> 4 per core) across the 8 cores, runs the
full module (fc1 -> attention -> fc2 -> GRU scan -> heads) per shard on
device, and concatenates shard outputs back to full shape.
"""
import numpy as np
import jax
import jax.numpy as jnp
from functools import partial

BS, TS, NE, ED = 32, 64, 64, 128
AE = 256
NH = 4
HD = AE // NH
RH = 256
NA = 8
NACT = 64
NEG = -1e10
NCORES = 8


def _attention(x1, pre_mask, in_w, out_w, out_b):
    qkv = x1 @ in_w.T
    q, k, v = jnp.split(qkv, 3, axis=-1)
    nb = x1.shape[0]
    q = q[:, :NA].reshape(nb, NA, NH, HD).transpose(0, 2, 1, 3)
    k = k.reshape(nb, NE, NH, HD).transpose(0, 2, 1, 3)
    v = v.reshape(nb, NE, NH, HD).transpose(0, 2, 1, 3)
    logits = jnp.einsum('bhqd,bhkd->bhqk', q, k) / jnp.sqrt(jnp.float32(HD))
    m = (pre_mask[:, None, :, :] != 0)
    dead = jnp.all(m, axis=-1, keepdims=True)
    logits = jnp.where(m, -jnp.inf, logits)
    logits = jnp.where(dead, 0.0, logits)
    w = jax.nn.softmax(logits, axis=-1)
    w = jnp.where(dead, 0.0, w)
    out = jnp.einsum('bhqk,bhkd->bhqd', w, v)
    out = out.transpose(0, 2, 1, 3).reshape(nb, NA, AE)
    return out @ out_w.T + out_b


def _gru_scan(x3, h0, wih, whh, bih, bhh):
    bs = x3.shape[0]
    xt = x3.transpose(1, 0, 2, 3).reshape(TS, bs * NA, RH)

    def step(h, x):
        gi = x @ wih.T + bih
        gh = h @ whh.T + bhh
        ir, iz, inn = jnp.split(gi, 3, axis=-1)
        hr, hz, hn = jnp.split(gh, 3, axis=-1)
        r = jax.nn.sigmoid(ir + hr)
        z = jax.nn.sigmoid(iz + hz)
        n = jnp.tanh(inn + r * hn)
        h_new = (1.0 - z) * n + z * h
        return h_new, h_new

    _, hs = jax.lax.scan(step, h0, xt)
    return hs.reshape(TS, bs, NA, RH).transpose(1, 0, 2, 3)


def _module(entities, obs_mask, entity_mask, hidden, use_attn,
            fc1_w, fc1_b, fc2_w, fc2_b, gru_wih, gru_whh, gru_bih, gru_bhh,
            fcq_w, fcq_b, fcpi_w, fcpi_b, attn=None):
    bs = entities.shape[0]
    b = bs * TS
    om = obs_mask.reshape(b, NE, NE)
    em = entity_mask.reshape(b, NE)
    agent_mask = em[:, :NA]
    if use_attn:
        e = entities.reshape(b, NE, ED)
        x1 = jax.nn.relu(e @ fc1_w.T + fc1_b)
        in_w, out_w, out_b = attn
        x2 = _attention(x1, om[:, :NA], in_w, out_w, out_b)
        x2 = jnp.where(agent_mask[:, :, None] != 0, 0.0, x2)
    else:
        # only the first NA entities of x1 are used downstream
        e = entities.reshape(b, NE, ED)[:, :NA]
        x2 = jax.nn.relu(e @ fc1_w.T + fc1_b)
    x3 = jax.nn.relu(x2 @ fc2_w.T + fc2_b).reshape(bs, TS, NA, RH)
    hs = _gru_scan(x3, hidden.reshape(-1, RH), gru_wih, gru_whh, gru_bih, gru_bhh)
    oq = hs @ fcq_w.T + fcq_b
    opi = hs @ fcpi_w.T + fcpi_b
    am = (agent_mask.reshape(bs, TS, NA, 1) != 0)
    oq = jnp.where(am, 0.0, oq)
    opi = jnp.where(am, NEG, opi)
    return oq, opi, hs


def _shard_fn(entities, obs_mask, entity_mask, hidden_q, hidden_pi, w):
    q, pi, h_q = _module(entities, obs_mask, entity_mask, hidden_q, True,
                         w['q_fc1_w'], w['q_fc1_b'], w['q_fc2_w'], w['q_fc2_b'],
                         w['q_gru_wih'], w['q_gru_whh'], w['q_gru_bih'], w['q_gru_bhh'],
                         w['q_fcq_w'], w['q_fcq_b'], w['q_fcpi_w'], w['q_fcpi_b'],
                         attn=(w['q_attn_in_w'], w['q_attn_out_w'], w['q_attn_out_b']))
    _, pi_avg, h_pi = _module(entities, obs_mask, entity_mask, hidden_pi, False,
                              w['p_fc1_w'], w['p_fc1_b'], w['p_fc2_w'], w['p_fc2_b'],
                              w['p_gru_wih'], w['p_gru_whh'], w['p_gru_bih'], w['p_gru_bhh'],
                              w['p_fcq_w'], w['p_fcq_b'], w['p_fcpi_w'], w['p_fcpi_b'])
    return q, pi, pi_avg, h_q, h_pi


_compiled = None


def _get_compiled():
    global _compiled
    if _compiled is None:
        devs = jax.devices()[:NCORES]
        mesh = jax.sharding.Mesh(np.array(devs), ('x',))
        P = jax.sharding.PartitionSpec
        sh_b = jax.sharding.NamedSharding(mesh, P('x'))
        sh_r = jax.sharding.NamedSharding(mesh, P())
        in_shardings = ((sh_b, sh_b, sh_b, sh_b, sh_b),
                        jax.tree_util.tree_map(lambda _: sh_r, {}))
        _compiled = (mesh, sh_b, sh_r)
    return _compiled


def kernel(**inputs):
    mesh, sh_b, sh_r = _get_compiled()
    batched_names = ('entities', 'obs_mask', 'entity_mask', 'hidden_q', 'hidden_pi')
    batched = [jax.device_put(np.ascontiguousarray(inputs[n]), sh_b)
               for n in batched_names]
    weights = {k: jax.device_put(np.ascontiguousarray(v), sh_r)
               for k, v in inputs.items() if k not in batched_names}

    fn = jax.jit(_shard_fn,
                 in_shardings=(sh_b, sh_b, sh_b, sh_b, sh_b,
                               jax.tree_util.tree_map(lambda _: sh_r, weights)),
                 out_shardings=(sh_b, sh_b, sh_b, sh_b, sh_b))
    out = fn(*batched, weights)
    out = jax.tree_util.tree_map(lambda a: np.asarray(a), out)
    return tuple(out)


if __name__ == '__main__':
    rng = np.random.default_rng(0)
    ins = dict(
        entities=rng.standard_normal((BS, TS, NE, ED), dtype=np.float32),
        obs_mask=rng.integers(0, 2, (BS, TS, NE, NE)).astype(np.int32),
        entity_mask=rng.integers(0, 2, (BS, TS, NE)).astype(np.int32),
        hidden_q=rng.standard_normal((BS, NA, RH), dtype=np.float32),
        hidden_pi=rng.standard_normal((BS, NA, RH), dtype=np.float32),
    )
    for p in ('q', 'p'):
        ins[f'{p}_fc1_w'] = rng.standard_normal((AE, ED), dtype=np.float32) * 0.05
        ins[f'{p}_fc1_b'] = rng.standard_normal((AE,), dtype=np.float32) * 0.05
        ins[f'{p}_fc2_w'] = rng.standard_normal((RH, AE), dtype=np.float32) * 0.05
        ins[f'{p}_fc2_b'] = rng.standard_normal((RH,), dtype=np.float32) * 0.05
        ins[f'{p}_gru_wih'] = rng.standard_normal((3 * RH, RH), dtype=np.float32) * 0.05
        ins[f'{p}_gru_whh'] = rng.standard_normal((3 * RH, RH), dtype=np.float32) * 0.05
        ins[f'{p}_gru_bih'] = rng.standard_normal((3 * RH,), dtype=np.float32) * 0.05
        ins[f'{p}_gru_bhh'] = rng.standard_normal((3 * RH,), dtype=np.float32) * 0.05
        ins[f'{p}_fcq_w'] = rng.standard_normal((NACT, RH), dtype=np.float32) * 0.05
        ins[f'{p}_fcq_b'] = rng.standard_normal((NACT,), dtype=np.float32) * 0.05
        ins[f'{p}_fcpi_w'] = rng.standard_normal((NACT, RH), dtype=np.float32) * 0.05
        ins[f'{p}_fcpi_b'] = rng.standard_normal((NACT,), dtype=np.float32) * 0.05
    ins['q_attn_in_w'] = rng.standard_normal((3 * AE, AE), dtype=np.float32) * 0.05
    ins['q_attn_out_w'] = rng.standard_normal((AE, AE), dtype=np.float32) * 0.05
    ins['q_attn_out_b'] = rng.standard_normal((AE,), dtype=np.float32) * 0.05
    out = kernel(**ins)
    print([o.shape for o in out])
